# revision 13
# baseline (speedup 1.0000x reference)
"""Trainium2 Bass kernel for the GNN descriptor problem (N=192 atoms).

Math: for each central atom i (cubic box, minimum-image convention):
  q_r[i,k]   = sum_j fc(r_ij) * r_ij^k                        (k=0..8)
  q_ang[i,n,l] = sum_{j,k} fc_ij fc_ik (r_ij r_ik)^n P_l(cos theta_jik)

The O(N^3) angular sum factorizes exactly into O(N^2) moments:
  P0: S_n^2;  P1: |V_n|^2;  P2: 1.5*||T_n||_F^2 - 0.5*S_n^2
  with S_n = sum_j fc r^n (= q_r[n]),  V_n,c = sum_j fc r^(n-1) dr_c,
  T_n,cc' = sum_j fc r^(n-2) dr_c dr_c'.

Sharding: 8 NeuronCores, 24 central atoms each (axis i), all 192 neighbors
local, no cross-device reduction; host concatenates the [24,18] shards.

On-chip layout: partitions 0-31/32-63/64-95 hold the x/y/z component planes
(rows 0-23 = atoms, 24-31 pad); free dim = neighbors j. The minimum-image
wrap is ONE custom-DVE op (ADD_RANGE_WRAP; per-partition -si*L shift rides
the scalar slot), so the host preps s*L directly. DR holds -dr; the sign
cancels (dr enters all outputs in even total powers).

Moments are accumulated as replicated-weight x product-plane ops: the
product planes SQ=dr^2, PROD=dr*rot(dr), R2DR=r^2*dr are built during
Vector idle slots (the Sqrt + Sin-table-reload shadow), then the nine
j-reductions are scalar_tensor_tensor accumulates into one MOM tile
(tensor_tensor_reduce crashes TRN2 here). V_2 = sum (w/r)*(r^2 dr) reuses
ps_t1, deleting the third replication matmul; r^2 is folded to all 96
partitions by the same PE matmul that used to double-fold it. t1/t2
replicate in separate matmuls so BT consumers unblock ~1us earlier.
Cross-component folds and replication run on the PE via selection-matrix
matmuls (two-SBUF-operand DVE ops require equal base partitions; PSUM
operands are exempt, so DVE reads PE results straight from PSUM). The
Scalar engine runs only Sqrt and Sin, warmed by dummy calls so LUT loads
overlap the input DMAs; the pi/RC prefactor is folded into the Sin scale
so the cutoff clamp is a single-op min scheduled before the reciprocal.

Framework trims: the Bass-constructor const-AP memsets + barrier and Tile's
kernel-tail barriers/sem-clear are patched out (drain kept — it guarantees
the output DMA lands); enable_partition_id=False drops the partition-id
input; the first input DMA rides the Scalar queue, which issues ~1us
earlier than Sync/GpSimd after the NRT preamble.
"""

import numpy as np

import concourse.bacc as bacc
import concourse.bass as bass_mod
import concourse.mybir as mybir
import concourse.tile as tile
from concourse.bass_utils import run_bass_kernel_spmd
from concourse.vector_clock import ScopedClock
from concourse.mybir import AluOpType as alu
from concourse.mybir import ActivationFunctionType as act
from concourse.dve_ops import ADD_RANGE_WRAP

N = 192
NCORES = 8
NI = N // NCORES  # 24
RC = 6.0
F32 = mybir.dt.float32
F32R = mybir.dt.float32r
PI = float(np.pi)

_cache = {}


def _build_program(box_diag):
    L = float(box_diag[0])
    orig_barrier = bass_mod.Bass.all_engine_barrier
    orig_memset = bass_mod.BassSharedVectorInterface.memset
    bass_mod.Bass.all_engine_barrier = lambda self, **kw: None
    bass_mod.BassSharedVectorInterface.memset = lambda self, ap, c: None
    try:
        nc = bacc.Bacc(
            "TRN2",
            target_bir_lowering=False,
            debug=False,
            enable_asserts=False,
            num_devices=NCORES,
            enable_partition_id=False,
        )
    finally:
        bass_mod.Bass.all_engine_barrier = orig_barrier
        bass_mod.BassSharedVectorInterface.memset = orig_memset

    def _drain_only(self, tick_clock, wait_clock):
        drain_inst = self.nc.sync.drain()
        wait_clock.add_sem_waits(
            drain_inst.ins, ScopedClock({None: tick_clock.global_clock})
        )
        popped = self.nc._tile_sem_poison_stack.pop()
        assert popped is self._sem_poison

    orig_dab = tile.TileContext._drain_and_barrier
    tile.TileContext._drain_and_barrier = _drain_only

    d_in1 = nc.dram_tensor("in1", [96, N + 4], F32, kind="ExternalInput")
    d_in2 = nc.dram_tensor("in2", [96, N + 192], F32R, kind="ExternalInput")
    d_out = nc.dram_tensor("out", [NI, 18], F32, kind="ExternalOutput")

    with tile.TileContext(nc) as tc:
        with tc.tile_pool(name="p", bufs=1) as pool, \
             tc.tile_pool(name="ps", bufs=1, space="PSUM") as ppool:
            t = lambda shape, name: pool.tile(shape, F32, name=name, tag=name)
            pt = lambda shape, name: ppool.tile(shape, F32, name=name, tag=name)

            V, S, G, T = nc.vector, nc.scalar, nc.gpsimd, nc.tensor

            # ---- inputs (two merged DMAs on separate queues) + constants ----
            IN1 = t([96, N + 4], "IN1")
            IN2 = pool.tile([96, N + 192], F32R, name="IN2", tag="IN2")
            nc.scalar.dma_start(out=IN1[:, :], in_=d_in1.ap())
            nc.sync.dma_start(out=IN2[:, :], in_=d_in2.ap())
            SJ = IN1[:, 0:N]       # s_j * L per c-block
            SC = IN1[:, N:N + 4]   # col 0: -s_i * L
            MASK = IN2[0:32, 0:N].bitcast(F32)
            SELF96 = IN2[:, N:N + 96]       # [96,96] triple fold lhsT
            SELF3 = IN2[:, N:N + 32]        # [96,32] fold lhsT (shared cols)
            SELR = IN2[0:32, N + 96:N + 192]  # [32,96] replicate lhsT

            cst = t([32, 3], "cst")
            V.memset(cst[:, 0:1], 1e-30)
            V.memset(cst[:, 1:2], PI / 2.0)
            V.memset(cst[:, 2:3], RC)

            # dummy activations: preload LUT tables; Sqrt last so the real
            # Sqrt is a table hit (walrus reloads on every function switch)
            dummy = t([1, 2], "dummy")
            S.activation(out=dummy[0:1, 1:2], in_=cst[0:1, 0:1], func=act.Sin,
                         bias=cst[0:1, 1:2], scale=-PI)
            S.activation(out=dummy[0:1, 0:1], in_=cst[0:1, 0:1], func=act.Sqrt,
                         bias=cst[0:1, 0:1])

            # ---- geometry: one wrap op -> (-dr) -> r^2 ----
            DR = t([96, N], "DR")
            SQ = pool.tile([96, N], F32R, name="SQ", tag="SQ")
            V._custom_dve(ADD_RANGE_WRAP, out=DR[:, :], in0=SJ,
                          s0=SC[:, 0:1], s1=L / 2.0, imm2=L)
            V.tensor_tensor(out=SQ[:, :], in0=DR[:, :], in1=DR[:, :], op=alu.mult)

            # r2 folded over c-blocks, replicated to all 96 partitions
            ps_r2 = pt([96, N], "ps_r2")
            T.matmul(ps_r2[:, :], SELF96, SQ[:, :], start=True, stop=True)

            RT = t([32, N], "RT")      # r
            RINV = t([32, N], "RINV")  # ~1/r
            S.activation(out=RT[:, :], in_=ps_r2[0:32, :], func=act.Sqrt,
                         bias=cst[:, 0:1])

            # rotated dr for off-diagonal pairs: blocks [dr_y | dr_z | dr_x]
            DROT = t([96, N], "DROT")
            G.tensor_copy(DROT[0:32, :], DR[32:64, :])
            G.tensor_copy(DROT[32:64, :], DR[64:96, :])
            G.tensor_copy(DROT[64:96, :], DR[0:32, :])

            # R2DR fills the Sqrt shadow (its dep, the PE fold, clears before
            # MIN's dep RT, so in-order DVE never stalls on it). No PROD
            # plane: off-diag channels reuse the BT products so nothing
            # pre-W waits on the slow Pool DROT copies.
            R2DR = t([96, N], "R2DR")
            X = t([32, N], "X")
            CX = t([32, N], "CX")
            W = pool.tile([32, N], F32R, name="W", tag="W")
            QACC = t([64, 8], "QACC")
            V.tensor_tensor(out=R2DR[:, :], in0=ps_r2[:, :], in1=DR[:, :],
                            op=alu.mult)
            V.tensor_scalar(out=X[:, :], in0=RT[:, :], scalar1=cst[:, 2:3],
                            scalar2=None, op0=alu.min)
            S.activation(out=CX[:, :], in_=X[:, :], func=act.Sin,
                         bias=cst[:, 1:2], scale=float(-PI / RC))
            V.reciprocal_approx_fast(out=RINV[:, :], in_=RT[:, :])
            # far pairs hit the clamp x=RC exactly; the HW Sin LUT returns
            # bit-exact -1.0 there (probed), so (cx+1)*mask is already zero
            # beyond the cutoff -- no explicit r<=rc indicator needed.
            V.scalar_tensor_tensor(out=W[:, :], in0=CX[:, :], scalar=1.0,
                                   in1=MASK, op0=alu.add, op1=alu.mult,
                                   accum_out=QACC[0:32, 0:1])

            def mul_accum(out, in0, in1, accum):
                V.scalar_tensor_tensor(out=out, in0=in0, scalar=0.0, in1=in1,
                                       op0=alu.bypass, op1=alu.mult,
                                       accum_out=accum)

            # ---- negative powers first (unblocks the t1/t2 rep matmuls) ----
            T12 = pool.tile([32, 2 * N], F32R, name="T12", tag="T12")  # [tm1 | tm2]
            V.tensor_tensor(out=T12[:, 0:N], in0=W[:, :].bitcast(F32), in1=RINV[:, :], op=alu.mult)
            V.tensor_tensor(out=T12[:, N:2 * N], in0=T12[:, 0:N].bitcast(F32), in1=RINV[:, :],
                            op=alu.mult)

            # ---- power chain: WA=[wk1|wk2], then *r2 twice pairwise ----
            WA = t([64, N], "WA")  # [wk1 | wk2]
            WB = t([64, N], "WB")  # [wk3 | wk4]
            WC = t([64, N], "WC")  # [wk5 | wk6]
            WD = t([64, N], "WD")  # [wk7 | wk8]
            mul_accum(WA[0:32, :], W[:, :].bitcast(F32), RT[:, :], QACC[0:32, 1:2])
            mul_accum(WA[32:64, :], W[:, :].bitcast(F32), ps_r2[0:32, :], QACC[0:32, 2:3])
            mul_accum(WB[0:64, :], WA[0:64, :], ps_r2[0:64, :], QACC[0:64, 3:4])
            mul_accum(WC[0:64, :], WB[0:64, :], ps_r2[0:64, :], QACC[0:64, 4:5])
            mul_accum(WD[0:64, :], WC[0:64, :], ps_r2[0:64, :], QACC[0:64, 5:6])

            # ---- replicated weights via PE rep3 (consumed straight from PSUM)
            ps_w = pt([96, N], "ps_w")
            ps_t1 = pt([96, N], "ps_t1")
            ps_t2 = pt([96, N], "ps_t2")
            T.matmul(ps_w[:, :], SELR, W[:, :], start=True, stop=True)
            T.matmul(ps_t1[:, :], SELR, T12[:, 0:N], start=True, stop=True)
            T.matmul(ps_t2[:, :], SELR, T12[:, N:2 * N], start=True, stop=True)

            # ---- moment accums (BT-hybrid); columns of MOM:
            # 0:3 = V_0,V_1,V_2; 3:6 = diag T_n; 6:9 = off-diag T_n.
            # BT_n = (w r^{n-1}) dr doubles as V_n's product and TD/TO input;
            # V_2 = (w/r)*(r^2 dr) rides ps_t1 with the shadowed R2DR plane.
            MOM = t([96, 9], "MOM")
            BT0 = t([96, N], "BT0")
            BT1 = t([96, N], "BT1")
            BT2 = t([96, N], "BT2")
            SCR = t([96, N], "SCR")
            mul_accum(BT2[:, :], ps_w[:, :], DR[:, :], MOM[:, 1:2])     # V_1
            mul_accum(SCR[:, :], BT2[:, :], DR[:, :], MOM[:, 5:6])      # TD_2
            mul_accum(SCR[:, :], BT2[:, :], DROT[:, :], MOM[:, 8:9])    # TO_2
            mul_accum(BT1[:, :], ps_t1[:, :], DR[:, :], MOM[:, 0:1])    # V_0
            mul_accum(SCR[:, :], BT1[:, :], DR[:, :], MOM[:, 4:5])      # TD_1
            mul_accum(SCR[:, :], BT1[:, :], DROT[:, :], MOM[:, 7:8])    # TO_1
            V.tensor_tensor(out=BT0[:, :], in0=ps_t2[:, :], in1=DR[:, :],
                            op=alu.mult)
            mul_accum(SCR[:, :], BT0[:, :], DR[:, :], MOM[:, 3:4])      # TD_0
            mul_accum(SCR[:, :], ps_t1[:, :], R2DR[:, :], MOM[:, 2:3])  # V_2
            mul_accum(SCR[:, :], BT0[:, :], DROT[:, :], MOM[:, 6:7])    # TO_0

            # ---- final combine, pipelined: fold V/TD columns as soon as
            # they are ready; TO^2 accumulates into the TD fold columns in
            # PSUM (start=False) so l=2 is a single STT afterwards. ----
            OT = t([32, 18], "OT")
            # q_r gathers (POOL): QACC cols = q0,q1,q2,(q3|q4),(q5|q6),(q7|q8)
            G.tensor_copy(OT[0:24, 0:3], QACC[0:24, 0:3])          # q0,q1,q2
            G.tensor_copy(OT[0:24, 3:8:2], QACC[0:24, 3:6])        # q3,q5,q7
            G.tensor_copy(OT[0:24, 4:9:2], QACC[32:56, 3:6])       # q4,q6,q8
            # l=0 on Pool (idle), off the DVE tail path
            SQS = t([32, 3], "SQS")
            G.tensor_tensor(out=SQS[0:24, :], in0=OT[0:24, 0:3],
                            in1=OT[0:24, 0:3], op=alu.mult)
            G.tensor_copy(OT[0:24, 9:16:3], SQS[0:24, :])

            SQ9 = t([96, 9], "SQ9")  # [sqV | 1.5*sqTd | 3*sqTo]
            ps_f = pt([32, 6], "ps_f")
            V.tensor_tensor(out=SQ9[:, 0:3], in0=MOM[:, 0:3], in1=MOM[:, 0:3],
                            op=alu.mult)
            V.scalar_tensor_tensor(out=SQ9[:, 3:6], in0=MOM[:, 3:6], scalar=1.5,
                                   in1=MOM[:, 3:6], op0=alu.mult, op1=alu.mult)
            T.matmul(ps_f[:, 0:3], SELF3.bitcast(F32), SQ9[:, 0:3], start=True, stop=True)
            T.matmul(ps_f[:, 3:6], SELF3.bitcast(F32), SQ9[:, 3:6], start=True, stop=False)
            V.scalar_tensor_tensor(out=SQ9[:, 6:9], in0=MOM[:, 6:9], scalar=3.0,
                                   in1=MOM[:, 6:9], op0=alu.mult, op1=alu.mult)
            T.matmul(ps_f[:, 3:6], SELF3.bitcast(F32), SQ9[:, 6:9], start=False, stop=True)

            # l=1: |V_n|^2 into OT cols 10,13,16 (GPSIMD cannot read PSUM)
            V.tensor_copy(OT[0:24, 10:17:3], ps_f[0:24, 0:3])
            # l=2: (1.5*td + 3*to) - 0.5*S_n^2 — single STT from PSUM
            V.scalar_tensor_tensor(out=OT[0:24, 11:18:3], in0=SQS[0:24, :],
                                   scalar=-0.5, in1=ps_f[0:24, 3:6],
                                   op0=alu.mult, op1=alu.add)

            nc.sync.dma_start(out=d_out.ap(), in_=OT[0:24, :],
                              single_packet=True)

    tile.TileContext._drain_and_barrier = orig_dab
    nc.compile()
    return nc


def _prep_inputs(R, box):
    """Host-side O(N) prep for the stacked layout."""
    box = np.asarray(box, dtype=np.float64)
    R = np.asarray(R, dtype=np.float32)
    box_inv = np.linalg.inv(box)
    s = (R.astype(np.float64) @ box_inv.T).astype(np.float32)  # [N,3]
    Ld = np.diag(box).astype(np.float32)

    in2 = np.zeros((96, N + 192), np.float32)
    for b in range(3):
        for bb in range(3):
            # SELF96: out partition block bb gets sum over c-blocks b
            in2[32 * b + np.arange(32), N + 32 * bb + np.arange(32)] = 1.0
        in2[np.arange(32), N + 96 + 32 * b + np.arange(32)] = 1.0      # SELR

    in_maps = []
    for core in range(NCORES):
        off = core * NI
        in1 = np.zeros((96, N + 4), np.float32)
        for c in range(3):
            in1[32 * c:32 * c + 32, 0:N] = (s[:, c] * Ld[c])[None, :]
            in1[32 * c:32 * c + NI, N] = -s[off:off + NI, c] * Ld[c]
            in1[32 * c + NI:32 * c + 32, N] = -0.5 * Ld[c]
        in2c = in2.copy()
        in2c[:NI, 0:N] = 0.5
        in2c[np.arange(NI), off + np.arange(NI)] = 0.0
        in_maps.append({"in1": in1, "in2": in2c})
    return in_maps


def run(R, Z, box, trace=False, **trace_kwargs):
    """Run on 8 NeuronCores; returns (out [N,18] f32, BassKernelResults)."""
    box = np.asarray(box)
    assert box.shape == (3, 3)
    if not np.allclose(box - np.diag(np.diag(box)), 0.0):
        raise NotImplementedError("kernel supports diagonal boxes only")
    bd = np.diag(box).astype(np.float64)
    assert np.allclose(bd, bd[0]), "wrap path assumes a cubic box"
    if "prog" not in _cache:
        _cache["prog"] = _build_program(bd)
    nc = _cache["prog"]

    in_maps = _prep_inputs(R, box)
    res = run_bass_kernel_spmd(nc, in_maps, core_ids=list(range(NCORES)),
                               trace=trace, **trace_kwargs)
    out = np.concatenate([res.results[c]["out"] for c in range(NCORES)], axis=0)
    return np.ascontiguousarray(out.astype(np.float32)), res


def kernel(R, Z, box):
    out, _ = run(R, Z, box)
    return out
